# revision 1
# baseline (speedup 1.0000x reference)
"""Trainium2 Bass kernel for nn_Condensation: 10 sequential masked-Gaussian-blur
composites over a [16,3,768,768] image, data-parallel over 8 NeuronCores.

v3 strategy (per core, 2 images = 6 image-channels):
  - State resident in SBUF as bf16 [128, 6, NB, 768]; only the h-blocks any
    drop touches (NB of 6) are loaded/stored; host copies untouched rows.
  - Host pre-packs img into per-(pair,half) fully-contiguous DRAM params so
    each DMA is 128 fat descriptors; f32->bf16 on host; per-(pair,block)
    output params so stores drain as soon as a block's last writer finishes.
  - All per-drop mask/kernel params prefetched upfront on the scalar queue.
  - Mask support shrunk to EPS=3e-3 and boxes capped so Wt<=256 (two
    overlapping 128-col chunks, WBn==2 always); all matmul bands, evictions
    and composites clipped to the true support.
  - Separable blur as two banded-matmul passes on TensorE (bf16, f32 PSUM);
    pass-A PSUM is a single 2-bank tile per (drop,pair) evicted in ONE op.
  - Composite via q-trick: q = out - om (tight, bf16 2x), then per h-block
    copy/mul/add ops chosen by a cost-calibrated greedy balancer across
    Vector/Scalar/GpSimd (including a direct-from-PSUM multiply option).
  - Warm-up matmuls during the initial DMA + a keep-warm dummy matmul per
    (drop,pair) so the PE HAM clock-gate stays at 2.4 GHz.
"""
import numpy as np
import ml_dtypes

NUM_DROPS = 10
MIN_R, MAX_R = 60.0, 80.0
BETA = 1.8
BLUR_RADII = [11.3535, 17.9381, 5.7966, 10.8586, 5.5301, 15.9075, 12.3225, 13.4871, 6.6639, 9.5413]


def _ksize(r):
    k = int(2 * r) + 1
    return k + 1 if k % 2 == 0 else k


KSIZES = [_ksize(r) for r in BLUR_RADII]
H = W = 768
B_TOTAL, C = 16, 3
N_CORES = 8
B_LOC = B_TOTAL // N_CORES          # 2 images per core
IC = B_LOC * C                      # 6 image-channels per core
NG = IC // 2                        # 3 pairs of image-channels
P = 128
EPS = 5e-3                          # mask support threshold (error-validated)

_bf16 = ml_dtypes.bfloat16


def _conv_matrix(sigma, ksize, n=768):
    """n x n matrix Kmat with blur_1d(x) = Kmat @ x, matching the reference
    (correlation with normalized gaussian, 'reflect' padding)."""
    half = (ksize - 1) * 0.5
    xs = np.linspace(-half, half, ksize)
    pdf = np.exp(-0.5 * (xs / np.float64(sigma)) ** 2)
    k1 = (pdf / pdf.sum()).astype(np.float32).astype(np.float64)
    pad = ksize // 2
    Kmat = np.zeros((n, n), dtype=np.float64)
    idx = np.arange(n)[:, None] + np.arange(ksize)[None, :] - pad
    idx = np.abs(idx)
    idx = np.where(idx >= n, 2 * n - 2 - idx, idx)
    np.add.at(Kmat, (np.repeat(np.arange(n), ksize), idx.ravel()),
              np.tile(k1, n))
    return Kmat.astype(np.float32)


class _Drop:
    pass


def _drop_meta(positions, radius):
    """Host-side per-drop geometry + tensors (shared across cores)."""
    pos = np.clip(np.asarray(positions, np.float32), -1.0, 1.0)
    rad = np.clip(np.asarray(radius, np.float32), MIN_R, MAX_R)
    hv = np.arange(H, dtype=np.float32)[:, None]
    wv = np.arange(W, dtype=np.float32)[None, :]
    s = float(np.sqrt((-np.log(EPS)) ** (1.0 / BETA)))
    s2 = s * s
    drops = []
    for j in range(NUM_DROPS):
        x0 = (pos[j, 0] + 1.0) / 2.0 * W
        y0 = (pos[j, 1] + 1.0) / 2.0 * H
        wr = rad[j]
        hr = wr * np.float32(0.8)
        ks = KSIZES[j]
        p = ks // 2
        d = _Drop()
        d.j, d.p = j, p
        h0 = max(0, int(np.floor(y0 - s * hr))) & ~1
        h1 = min(H, (int(np.ceil(y0 + s * hr)) + 2) & ~1)
        w0 = max(0, int(np.floor(x0 - s * wr))) & ~1
        w1 = min(W, (int(np.ceil(x0 + s * wr)) + 2) & ~1)
        # psum tiles are one 2KB bank per j: span <= 256 and Wt <= 256
        while h1 - h0 > 256:
            if y0 - h0 > h1 - y0:
                h0 += 2
            else:
                h1 -= 2
        wcap = 256 - 2 * p - 2
        while w1 - w0 > wcap:
            if x0 - w0 > w1 - x0:
                w0 += 2
            else:
                w1 -= 2
        d.h0, d.h1, d.w0, d.w1 = h0, h1, w0, w1
        d.span = h1 - h0
        d.Wr = w1 - w0
        d.HB0 = h0 // P
        HB1 = (h1 + P - 1) // P
        d.HBn = HB1 - d.HB0
        d.HBs = d.HB0 * P
        d.HBw = d.HBn * P
        d.voff = h0 - d.HBs            # vt column offset of h0 within blocks
        wa = max(0, w0 - p) & ~1
        wb = min(W, (w1 + p + 1) & ~1)
        d.wa, d.wb = wa, wb
        d.Wt = wb - wa
        assert d.Wt <= 256 and d.span <= 256
        d.WBn = (d.Wt + P - 1) // P
        assert d.WBn == 2
        d.cstarts = [wa + P * i for i in range(d.WBn - 1)] + [wb - P]

        # pass A bands per k-block: output h' range (relative to h0)
        d.bandsA = []
        for k in range(d.HBn):
            a = max(0, d.HBs + P * k - p - h0)
            b = min(d.span, d.HBs + P * (k + 1) + p - h0)
            d.bandsA.append((a, b))

        # per h-block composite w-range [wl, wr) from the ellipse extent
        d.hbw = []
        for hb in range(d.HBn):
            ra = max(h0, d.HBs + P * hb)
            rb = min(h1, d.HBs + P * (hb + 1))
            if ra - 1 < y0 < rb:
                dh = 0.0
            else:
                dh = min(abs(ra - y0), abs(rb - 1 - y0))
            half = wr * np.sqrt(max(0.0, s2 - (dh / hr) ** 2))
            wl = max(w0, (int(np.floor(x0 - half)) - 2) & ~1)
            wr_ = min(w1, (int(np.ceil(x0 + half)) + 4) & ~1)
            wr_ = max(wr_, wl + 2)
            d.hbw.append((wl, wr_))

        # pass B bands per (hb, wc): output w' range (absolute), or None
        d.bandsB = []
        for hb in range(d.HBn):
            wl, wr_ = d.hbw[hb]
            row = []
            for wc in range(d.WBn):
                c = d.cstarts[wc]
                a = max(wl, c - p)
                b = min(wr_, c + P + p)
                row.append((a - w0, b - w0) if b > a else None)
            d.bandsB.append(row)

        # mask over [HBw rows] x [wa:wb], zero outside support
        dd = (hv[d.HBs:d.HBs + d.HBw] - y0) ** 2 / hr ** 2 + \
             (wv[:, wa:wb] - x0) ** 2 / wr ** 2
        m = np.clip(np.exp(-(dd.astype(np.float32) ** np.float32(BETA)) + np.float32(1e-10)), 0.0, 1.0)
        m = np.where(dd <= np.float32(s2), m, 0.0).astype(np.float32)
        mz = np.zeros_like(m)
        for hb in range(d.HBn):
            ra = max(h0, d.HBs + P * hb) - d.HBs
            rb = min(h1, d.HBs + P * (hb + 1)) - d.HBs
            wl, wr_ = d.hbw[hb]
            mz[ra:rb, wl - wa:wr_ - wa] = m[ra:rb, wl - wa:wr_ - wa]
        m1 = np.ascontiguousarray(
            mz.reshape(d.HBn, P, d.Wt).transpose(1, 0, 2)).astype(_bf16)
        d.m_np = np.ascontiguousarray(
            np.broadcast_to(m1[:, None], (P, 2, d.HBn, d.Wt)))

        MT = _conv_matrix(BLUR_RADII[j], ks).T    # MT[src, dst]
        kv = np.zeros((P, d.HBn, d.span), np.float32)
        for k in range(d.HBn):
            kv[:, k, :] = MT[d.HBs + P * k:d.HBs + P * (k + 1), h0:h1]
        d.kv_np = np.ascontiguousarray(kv.astype(_bf16))
        kh = np.zeros((P, d.WBn, d.Wr), np.float32)
        for wc in range(d.WBn):
            c = d.cstarts[wc]
            kh[:, wc, :] = MT[c:c + P, w0:w1]
        # the last w-chunk may overlap the previous one: zero duplicated rows
        if d.WBn >= 2:
            dup = wa + P * (d.WBn - 1) - d.cstarts[-1]
            if dup > 0:
                kh[:dup, d.WBn - 1, :] = 0.0
        d.kh_np = np.ascontiguousarray(kh.astype(_bf16))
        drops.append(d)
    return drops


class _Balancer:
    """Greedy static load-balancer across Vector/Scalar/GpSimd with
    HW-calibrated per-op costs (ns)."""

    def __init__(self, nc):
        self.nc = nc
        self.load = {'V': 0.0, 'S': 0.0, 'G': 0.0}

    def _pick(self, costs):
        eng, c = min(costs, key=lambda ec: self.load[ec[0]] + ec[1])
        self.load[eng] += c
        return eng

    def tt(self, op, out, a, b, fd):
        eng = self._pick([('V', fd / 2 / 0.96 + 170), ('G', fd * 2.2 + 180)])
        e = self.nc.vector if eng == 'V' else self.nc.gpsimd
        getattr(e, 'tensor_' + op)(out, a, b)

    def copy(self, out, src, fd):
        eng = self._pick([('V', fd / 2 / 0.96 + 190), ('S', fd / 1.2 + 160)])
        if eng == 'V':
            self.nc.vector.tensor_copy(out, src)
        else:
            self.nc.scalar.copy(out=out, in_=src)

    def bsh_mul(self, psb_sl, bsh_sl, m_sl, t2_sl, fd):
        """t2 = m * psb, either via {S|V} copy + {V|G} mul, or V direct."""
        cV, cS = fd / 2 / 0.96 + 190, fd / 1.2 + 160
        mV, mG = fd / 2 / 0.96 + 170, fd * 2.2 + 180
        dV = fd / 0.96 + 190
        best, opt = None, None
        for tag, deltas in [('SV', (('S', cS), ('V', mV))),
                            ('SG', (('S', cS), ('G', mG))),
                            ('VV', (('V', cV), ('V', mV))),
                            ('VG', (('V', cV), ('G', mG))),
                            ('D', (('V', dV),))]:
            tmp = dict(self.load)
            for e, c in deltas:
                tmp[e] += c
            key = (max(tmp.values()), sum(tmp.values()))
            if best is None or key < best:
                best, opt = key, (tag, deltas)
        tag, deltas = opt
        for e, c in deltas:
            self.load[e] += c
        if tag == 'D':
            self.nc.vector.tensor_mul(t2_sl, m_sl, psb_sl)
        else:
            if tag[0] == 'S':
                self.nc.scalar.copy(out=bsh_sl, in_=psb_sl)
            else:
                self.nc.vector.tensor_copy(bsh_sl, psb_sl)
            e = self.nc.vector if tag[1] == 'V' else self.nc.gpsimd
            e.tensor_mul(t2_sl, m_sl, bsh_sl)


def _build_program(drops, NB):
    from contextlib import ExitStack
    from concourse import bacc, tile, mybir

    f32 = mybir.dt.float32
    bf16 = mybir.dt.bfloat16

    nc = bacc.Bacc("TRN2", target_bir_lowering=False, debug=False,
                   num_devices=N_CORES)
    imgs_d = [[nc.declare_dram_parameter(f"i{g}h{h}", [P, 2, 2, W], bf16, False)
               for h in range(2)] for g in range(NG)]
    outs_d = [[nc.declare_dram_parameter(f"o{ic}h{h}", [P, 1 if h == 0 else NB - 1, W], bf16, True)
               for h in range(2)] for ic in range(IC)]
    # drop params batched into two padded chunks (drops 0-1 hot, 2-9 bulk)
    # so only 6 DMA instructions hit the queue instead of 30
    KMAX, WMAX = 3, 256
    chunks = [(0, 2), (2, NUM_DROPS)]
    kvoffs, khoffs = [], []
    kvlen = [0, 0]
    khlen = [0, 0]
    for dj, d in enumerate(drops):
        ci = 0 if dj < 2 else 1
        kvoffs.append(kvlen[ci])
        khoffs.append(khlen[ci])
        kvlen[ci] += d.HBn * d.span
        khlen[ci] += d.WBn * d.Wr
    pchunks = []
    for ci, (d0, d1) in enumerate(chunks):
        nd = d1 - d0
        pchunks.append((
            nc.declare_dram_parameter(f"mc{ci}", [P, nd, 2, KMAX, WMAX], bf16, False),
            nc.declare_dram_parameter(f"kvc{ci}", [P, kvlen[ci]], bf16, False),
            nc.declare_dram_parameter(f"khc{ci}", [P, khlen[ci]], bf16, False)))

    bal = _Balancer(nc)
    HBWMAX = max(d.HBw for d in drops)

    # drop dependency DAG: read = h-blocks x [wa,wb), write = [h0,h1)x[w0,w1)
    def _ovl(a0, a1, b0, b1):
        return max(a0, b0) < min(a1, b1)

    def _dep(i, j):
        di, dj_ = drops[i], drops[j]
        ri = (di.HBs, di.HBs + di.HBw, di.wa, di.wb)
        wi = (di.h0, di.h1, di.w0, di.w1)
        rj = (dj_.HBs, dj_.HBs + dj_.HBw, dj_.wa, dj_.wb)
        wj = (dj_.h0, dj_.h1, dj_.w0, dj_.w1)
        for (a, b) in ((wi, rj), (ri, wj), (wi, wj)):
            if _ovl(a[0], a[1], b[0], b[1]) and _ovl(a[2], a[3], b[2], b[3]):
                return True
        return False

    level = [0] * NUM_DROPS
    for j in range(NUM_DROPS):
        for i in range(j):
            if _dep(i, j):
                level[j] = max(level[j], level[i] + 1)
    waves = []
    for lv in range(max(level) + 1):
        waves.append([dj for dj in range(NUM_DROPS) if level[dj] == lv])
    print("drop waves:", waves)

    with tile.TileContext(nc) as tc, ExitStack() as ctx:
        outp = ctx.enter_context(tc.tile_pool(name="out_state", bufs=1))
        out_s = outp.tile([P, IC, NB, W], bf16, name="state", tag="state")
        dp = ctx.enter_context(tc.tile_pool(name="dropin", bufs=1))
        omp = ctx.enter_context(tc.tile_pool(name="omq", bufs=8))
        vtp = ctx.enter_context(tc.tile_pool(name="vts", bufs=8))
        bshp = ctx.enter_context(tc.tile_pool(name="bsh", bufs=8))
        ppa = ctx.enter_context(tc.tile_pool(name="psa", bufs=2, space="PSUM"))
        ppb = ctx.enter_context(tc.tile_pool(name="psb", bufs=4, space="PSUM"))

        # ---- PE warm-up: ~18 matmuls on a zeroed tile span the HAM window
        wt = dp.tile([P, 512], bf16, tag="warm")
        nc.gpsimd.memset(wt[:], 0)
        warm = ppa.tile([P, 2, 2, 256], f32, tag="psa")
        for i in range(46):
            nc.tensor.matmul(warm[:, 0, 0, 0:256], lhsT=wt[:, 0:P],
                             rhs=wt[:, 0:256], start=True, stop=True)
        # pre-zero the vt ring so pass-B stationaries never read NaN garbage
        for i in range(8):
            v0 = vtp.tile([P, 2, 2, HBWMAX], bf16, tag="vt", bufs=8)
            (nc.vector if i % 2 else nc.gpsimd).memset(v0[:], 0)

        # ---- loads, all on the sync queue (no compute engine behind it):
        # img first halves, hot params (drops 0-1), img second halves, rest
        ptiles = []
        for ci, (d0, d1) in enumerate(chunks):
            nd = d1 - d0
            ptiles.append((
                dp.tile([P, nd, 2, KMAX, WMAX], bf16, tag=f"mc{ci}", name=f"mc{ci}"),
                dp.tile([P, kvlen[ci]], bf16, tag=f"kvc{ci}", name=f"kvc{ci}"),
                dp.tile([P, khlen[ci]], bf16, tag=f"khc{ci}", name=f"khc{ci}")))
        for ti, (t, pd) in enumerate(zip(ptiles[0], pchunks[0])):
            (nc.sync if ti == 0 else nc.scalar).dma_start(out=t[:], in_=pd.ap()[:])
        nc.sync.dma_start(out=out_s[:, 0:2, 0:2, :], in_=imgs_d[0][0].ap()[:])
        for g in range(1, NG):
            nc.sync.dma_start(out=out_s[:, 2 * g:2 * g + 2, 0:2, :],
                              in_=imgs_d[g][0].ap()[:])
        for g in range(NG):
            nc.sync.dma_start(out=out_s[:, 2 * g:2 * g + 2, 2:4, :],
                              in_=imgs_d[g][1].ap()[:])
        for t, pd in zip(ptiles[1], pchunks[1]):
            nc.sync.dma_start(out=t[:], in_=pd.ap()[:])
        # ---- drops
        for dj, d in enumerate(drops):
            ci = 0 if dj < 2 else 1
            i = dj - chunks[ci][0]
            mt, kvt, kht = ptiles[ci]
            kvo, kho = kvoffs[dj], khoffs[dj]
            for g in range(NG):
                jb = 0
                sl = out_s[:, 2 * g:2 * g + 2, d.HB0:d.HB0 + d.HBn, d.wa:d.wb]
                slq = out_s[:, 2 * g:2 * g + 2, d.HB0:d.HB0 + d.HBn, d.w0:d.w1]
                om = omp.tile([P, 2, d.HBn, d.Wt], bf16, tag="om")
                bal.tt('mul', om[:], mt[:, i, 0:2, 0:d.HBn, 0:d.Wt], sl,
                       2 * d.HBn * d.Wt)
                q = omp.tile([P, 2, d.HBn, d.Wr], bf16, tag="q")
                bal.tt('sub', q[:], slq,
                       om[:, :, :, d.w0 - d.wa:d.w0 - d.wa + d.Wr],
                       2 * d.HBn * d.Wr)
                # pass A: vT[w-chunk, h'] banded over the support
                psa = ppa.tile([P, 2, 2, 256], f32, tag="psa")
                for wc in range(d.WBn):
                    coff = d.cstarts[wc] - d.wa
                    for jj in range(2):
                        for k in range(d.HBn):
                            a, b = d.bandsA[k]
                            nc.tensor.matmul(
                                psa[:, jj, wc, a:b],
                                lhsT=om[:, jb + jj, k, coff:coff + P],
                                rhs=kvt[:, kvo + k * d.span + a:kvo + k * d.span + b],
                                start=(k == 0), stop=(k == d.HBn - 1))
                vt = vtp.tile([P, 2, 2, d.HBw], bf16, tag="vt", bufs=8)
                bal.copy(vt[:, :, :, d.voff:d.voff + d.span],
                         psa[:, :, :, 0:d.span], 4 * d.span)
                # pass B + composite per h'-block
                for hb in range(d.HBn):
                    psb = ppb.tile([P, 2, 256], f32, tag="psb")
                    live = [(wc, ab) for wc, ab in enumerate(d.bandsB[hb]) if ab]
                    for jj in range(2):
                        for li, (wc, (a, b)) in enumerate(live):
                            nc.tensor.matmul(
                                psb[:, jj, a:b],
                                lhsT=vt[:, jj, wc, hb * P:(hb + 1) * P],
                                rhs=kht[:, kho + wc * d.Wr + a:kho + wc * d.Wr + b],
                                start=(li == 0), stop=(li == len(live) - 1))
                    wl, wr_ = d.hbw[hb]
                    wid = wr_ - wl
                    acol = wl - d.w0
                    Bsh = bshp.tile([P, 2, 256], bf16, tag="Bs")
                    t2 = bshp.tile([P, 2, 256], bf16, tag="t2")
                    bal.bsh_mul(psb[:, :, acol:acol + wid],
                                Bsh[:, :, acol:acol + wid],
                                mt[:, i, 0:2, hb, wl - d.wa:wr_ - d.wa],
                                t2[:, :, 0:wid], 2 * wid)
                    osl = out_s[:, 2 * g:2 * g + 2, d.HB0 + hb, wl:wr_]
                    bal.tt('add', osl,
                           q[:, jb:jb + 2, hb, wl - d.w0:wr_ - d.w0],
                           t2[:, :, 0:wid], 2 * wid)

        # ---- stores: per ic, block 0 (final after d8) then blocks 1..NB
        qeng = [nc.sync, nc.scalar]
        for ic in range(IC):
            qeng[ic % 2].dma_start(out=outs_d[ic][0].ap()[:],
                                   in_=out_s[:, ic, 0:1, :])
        for ic in range(IC):
            qeng[ic % 2].dma_start(out=outs_d[ic][1].ap()[:],
                                   in_=out_s[:, ic, 1:NB, :])

    nc.compile()
    print("balancer loads (us):",
          {k: round(v / 1000, 1) for k, v in bal.load.items()})
    return nc


_CACHE = {}


def _get_program(positions, radius):
    key = (np.asarray(positions, np.float32).tobytes(),
           np.asarray(radius, np.float32).tobytes())
    if key not in _CACHE:
        drops = _drop_meta(positions, radius)
        NB = max(d.HB0 + d.HBn for d in drops)
        _CACHE[key] = (_build_program(drops, NB), drops, NB)
    return _CACHE[key]


def kernel(img, positions, radius, _want_trace=False, **_kw):
    from concourse.bass_utils import run_bass_kernel_spmd
    img = np.asarray(img, np.float32)
    assert img.shape == (B_TOTAL, C, H, W)
    nc, drops, NB = _get_program(positions, radius)

    # pack to SBUF layout [p, pair(2), blk, w] per (core, pair, half), bf16
    imgb = img[:, :, 0:NB * P, :].astype(_bf16)
    packed = np.ascontiguousarray(
        imgb.reshape(N_CORES, B_LOC * C, NB, P, W).transpose(0, 3, 1, 2, 4))
    KMAX, WMAX = 3, 256
    base = {}
    for ci, (d0, d1) in enumerate([(0, 2), (2, NUM_DROPS)]):
        nd = d1 - d0
        mc = np.zeros((P, nd, 2, KMAX, WMAX), _bf16)
        for i, d in enumerate(drops[d0:d1]):
            mc[:, i, :, 0:d.HBn, 0:d.Wt] = d.m_np
        base[f"mc{ci}"] = mc
        base[f"kvc{ci}"] = np.ascontiguousarray(np.concatenate(
            [d.kv_np.reshape(P, -1) for d in drops[d0:d1]], axis=1))
        base[f"khc{ci}"] = np.ascontiguousarray(np.concatenate(
            [d.kh_np.reshape(P, -1) for d in drops[d0:d1]], axis=1))
    in_maps = []
    for i in range(N_CORES):
        mp = dict(base)
        for g in range(NG):
            for hh in range(2):
                mp[f"i{g}h{hh}"] = np.ascontiguousarray(
                    packed[i][:, 2 * g:2 * g + 2, 2 * hh:2 * hh + 2, :])
        in_maps.append(mp)
    res = run_bass_kernel_spmd(nc, in_maps, core_ids=list(range(N_CORES)),
                               trace=_want_trace)
    out = img.copy()
    for i in range(N_CORES):
        blk = np.empty((P, IC, NB, W), _bf16)
        for ic in range(IC):
            blk[:, ic, 0:1, :] = res.results[i][f"o{ic}h0"]
            blk[:, ic, 1:NB, :] = res.results[i][f"o{ic}h1"]
        out[B_LOC * i:B_LOC * (i + 1), :, 0:NB * P, :] = blk.transpose(
            1, 2, 0, 3).reshape(B_LOC, C, NB * P, W).astype(np.float32)
    if _want_trace:
        return out, res
    return out

